# revision 31
# baseline (speedup 1.0000x reference)
"""Trainium2 Bass kernel: modulated deformable conv 3x3 (DCNv2) + BN(eval)
+ ReLU.  B=4, C=O=256, H=W=64, distributed over 8 NeuronCores.

Sharding: core i -> batch b = i//2, image row-half = i%2 (32 rows). Each core
computes out[b, :, h0:h0+32, :] fully.

Per-core pipeline (all x-dependent compute on device):
  - xpad [C,74*74] fp16 zero-padded image (pad P=5) via casting SWDGE DMA.
  - offset conv om[27,2048] from a 34-row halo slice (9 shifted matmuls).
  - om -> omT [hw_part, 27]; DVE computes bilinear corner weights and two
    int16 PAIR indices per (sample, tap) (top q, bottom q+W2).  xt1 is
    stored ONCE as [5504, 256] fp16; the gather uses an overlapping AP
    (elem_size=512, elem_step=256) so one 1KB element covers rows q, q+1
    (both x-corners).
  - gathers are prepare_only SWDGE descriptor preps + trigger_dma on 4
    queues round-robin: the Pool engine only generates descriptors while
    the 16 DMA engines move data in the background; DVE waits on the
    per-queue completion semaphore before combining.
  - per (s-block, tap): 2 gathers -> [sample, 2*256c] fp16 x2; DVE
    combines 4 corners via 7 wide tensor_tensor ops with stride-0
    broadcast corner weights (w[p,chunk] broadcast over the 256-channel
    axis); PE transposes V^T -> V[c,s]; matmuls accumulate out^T[s,o]
    over (tap,cc) in PSUM; bias via K=1 matmul; ReLU; PE transpose to
    [o,s]; store.
BN is folded on device: W' = W * (gamma*rsqrt(var+eps)) ; b' = s*(bias-mean)+beta.
"""

import numpy as np

import concourse.bass as bass
import concourse.bacc as bacc
import concourse.mybir as mybir
import concourse.tile as tile
from concourse import bass_utils, library_config

F32 = mybir.dt.float32
F16 = mybir.dt.float16
I16 = mybir.dt.int16
I32 = mybir.dt.int32
AF = mybir.ActivationFunctionType
ALU = mybir.AluOpType

B, C, O, H, W = 4, 256, 256, 64, 64
K = 9
P = 5
W2 = H + 2 * P            # 74
NQ = W2 * W2              # 5476
NQP = 5504                # 43*128
NQA = 5632                # quad-tensor alloc rows (>= NQP+76)
QSH = 75                  # quad alloc shift: idx = q + QSH
HH = 32                   # rows per core
S = HH * W                # 2048 samples per core
CC = C // 128             # 2
OCC = O // 128            # 2
NSB = 2                   # sample blocks
SB = S // NSB             # 1024 samples / block
CHB = 8                   # chunks per block
NI = SB                   # idxs per gather call (quad rows) = 1024
NG = NSB * K              # gather calls = 18
EPS = 1e-5
N_CORES = 8
NQUEUES = 4

_NC_CACHE = {}


def build_nc():
    nc = bacc.Bacc("TRN2", target_bir_lowering=False, debug=False,
                   num_devices=N_CORES, num_swdge_queues=NQUEUES)

    x_in = nc.dram_tensor("x_b", [C, H, W], F32, kind="ExternalInput")
    xhalo = nc.dram_tensor("xhalo", [C, 34, W], F32, kind="ExternalInput")
    w_t = nc.dram_tensor("w_t", [CC, 128, K * O], F32, kind="ExternalInput")
    woff_t = nc.dram_tensor("woff_t", [CC, 128, K * 27], F32, kind="ExternalInput")
    b_off_in = nc.dram_tensor("b_off", [27, 1], F32, kind="ExternalInput")
    bnvec = nc.dram_tensor("bnvec", [1, 5 * O], F32, kind="ExternalInput")
    baseC = nc.dram_tensor("baseC", [128, 16 * 32], F32, kind="ExternalInput")
    ident32 = nc.dram_tensor("ident32", [128, 128], F32, kind="ExternalInput")
    ident16 = nc.dram_tensor("ident16", [128, 128], F16, kind="ExternalInput")
    ones16 = nc.dram_tensor("ones16", [1, 128], F16, kind="ExternalInput")

    out_d = nc.dram_tensor("out_c", [O, S], F32, kind="ExternalOutput")

    with tile.TileContext(nc) as tc:
        _build(nc, tc, x_in, xhalo, w_t, woff_t, b_off_in, bnvec, baseC,
               ident32, ident16, ones16, out_d)
    nc.compile()
    return nc


def _build(nc, tc, x_in, xhalo, w_t, woff_t, b_off_in, bnvec, baseC,
           ident32, ident16, ones16, out_d):
    from contextlib import ExitStack

    with ExitStack() as top:
        pers = top.enter_context(tc.tile_pool(name="pers", bufs=1))
        dram = top.enter_context(tc.tile_pool(name="dram", bufs=1, space="DRAM"))
        xt1q_t = dram.tile([NQA, 1024], F16, name="xt1q_scr", tag="xt1q")
        idxs_t = dram.tile([NSB, K, 16, 64], I16, name="idx_scr", tag="idxs")
        ph1_cm = tc.tile_pool(name="ph1", bufs=1)
        ph1 = ph1_cm.__enter__()

        # ------------- constants -------------
        id32 = pers.tile([128, 128], F32)
        nc.sync.dma_start(out=id32[:], in_=ident32.ap())
        id16 = pers.tile([128, 128], F16)
        nc.sync.dma_start(out=id16[:], in_=ident16.ap())
        one16 = pers.tile([1, 128], F16)
        nc.sync.dma_start(out=one16[:], in_=ones16.ap())
        base_t = pers.tile([128, 16, 32], F32)
        nc.sync.dma_start(out=base_t[:], in_=baseC.ap().rearrange("p (a b) -> p a b", a=16))
        boff_t = pers.tile([27, 1], F32)
        nc.sync.dma_start(out=boff_t[:], in_=b_off_in.ap())

        # ------------- BN fold -------------
        bn_t = pers.tile([1, 5 * O], F32)
        nc.sync.dma_start(out=bn_t[:], in_=bnvec.ap())
        gam = bn_t[:, 0:O]; bet = bn_t[:, O:2 * O]; rmn = bn_t[:, 2 * O:3 * O]
        rvr = bn_t[:, 3 * O:4 * O]; bia = bn_t[:, 4 * O:5 * O]
        sq = pers.tile([1, O], F32)
        nc.vector.tensor_scalar(sq[:], rvr, float(EPS), None, ALU.add)
        nc.scalar.activation(sq[:], sq[:], AF.Sqrt)
        sfac = pers.tile([1, O], F32)
        nc.vector.reciprocal(sfac[:], sq[:])
        nc.vector.tensor_tensor(out=sfac[:], in0=sfac[:], in1=gam, op=ALU.mult)
        bpr = pers.tile([1, O], F32)
        nc.vector.tensor_tensor(out=bpr[:], in0=bia, in1=rmn, op=ALU.subtract)
        nc.vector.tensor_tensor(out=bpr[:], in0=bpr[:], in1=sfac[:], op=ALU.mult)
        nc.vector.tensor_tensor(out=bpr[:], in0=bpr[:], in1=bet, op=ALU.add)
        bprow16 = pers.tile([1, O], F16)
        nc.vector.tensor_copy(bprow16[:], bpr[:])
        sbc = pers.tile([128, O], F32)
        ones32 = pers.tile([1, 128], F32)
        nc.vector.memset(ones32[:], 1.0)
        with tc.tile_pool(name="bcp", bufs=1, space="PSUM") as bcp:
            bc_ps = bcp.tile([128, O], F32)
            nc.tensor.matmul(bc_ps[:], ones32[:], sfac[:], start=True, stop=True)
            nc.vector.tensor_copy(sbc[:], bc_ps[:])

        # ------------- weights -------------
        wmain = []
        for cc in range(CC):
            wmain.append(pers.tile([128, K * O], F16, name=f"wmain{cc}", tag=f"wmain{cc}"))
        woff16 = []
        for cc in range(CC):
            woff16.append(ph1.tile([128, K * 27], F16, name=f"woff{cc}", tag=f"woff{cc}"))

        with tc.tile_pool(name="wtmp", bufs=1) as wtmp:
            for cc in range(CC):
                wr = wtmp.tile([128, K * O], F32, name=f"wr{cc}", tag="wr")
                nc.sync.dma_start(out=wr[:], in_=w_t.ap()[cc])
                for k in range(K):
                    nc.vector.tensor_tensor(out=wmain[cc][:, k * O:(k + 1) * O],
                                            in0=wr[:, k * O:(k + 1) * O],
                                            in1=sbc[:], op=ALU.mult)
                wo = wtmp.tile([128, K * 27], F32, name=f"wo{cc}", tag="wo")
                nc.sync.dma_start(out=wo[:], in_=woff_t.ap()[cc])
                nc.vector.tensor_copy(woff16[cc][:], wo[:])

        # ------------- xom (halo, fp16) first: om conv is the critical chain
        xom = []
        for cc in range(CC):
            t = ph1.tile([128, 34 * W2], F16, name=f"xom{cc}", tag=f"xom{cc}")
            v = t[:].rearrange("p (h w) -> p h w", w=W2)
            nc.vector.memset(v[:, :, 0:P], 0.0)
            nc.vector.memset(v[:, :, P + W:W2], 0.0)
            nc.gpsimd.dma_start(out=v[:, :, P:P + W],
                                in_=xhalo.ap()[cc * 128:(cc + 1) * 128])
            xom.append(t)
        # ------------- xpad (full, fp16); memset pad regions only ---------
        xpad = []
        for cc in range(CC):
            t = ph1.tile([128, NQP], F16, name=f"xpad{cc}", tag=f"xpad{cc}")
            nc.vector.memset(t[:, 0:P * W2], 0.0)                 # top pad rows
            nc.vector.memset(t[:, (P + H) * W2:NQP], 0.0)         # bottom + tail
            v = t[:, 0:NQ].rearrange("p (h w) -> p h w", w=W2)
            nc.vector.memset(v[:, P:P + H, 0:P], 0.0)             # left cols
            nc.vector.memset(v[:, P:P + H, P + W:W2], 0.0)        # right cols
            nc.gpsimd.dma_start(out=v[:, P:P + H, P:P + W],
                                in_=x_in.ap()[cc * 128:(cc + 1) * 128])
            xpad.append(t)

        # ------------- offset conv: om [27, 2048] -------------
        om_sb = ph1.tile([27, S], F32)
        omT = ph1.tile([128, 16, 32], F32)
        with tc.tile_pool(name="omps", bufs=1, space="PSUM") as omps:
            om_ps = omps.tile([27, S], F32, name="om_ps", tag="om_ps")
            for bk in range(4):           # 4 banks of 512 (8 rows x 64)
                for cc in range(CC):
                    for t9 in range(K):
                        ty, tx = t9 // 3, t9 % 3
                        rhs = xom[cc][:].rearrange("p (h w) -> p h w", w=W2)[
                            :, bk * 8 + ty: bk * 8 + ty + 8,
                            P - 1 + tx: P - 1 + tx + W]
                        nc.tensor.matmul(om_ps[:, bk * 512:(bk + 1) * 512],
                                         woff16[cc][:, t9 * 27:(t9 + 1) * 27], rhs,
                                         start=(cc == 0 and t9 == 0),
                                         stop=(cc == CC - 1 and t9 == K - 1))
            nc.scalar.activation(om_sb[:], om_ps[:], AF.Identity,
                                 bias=boff_t[:, 0:1])

            # ------------- omT [128, 16, 32] -------------
            omT_ps = omps.tile([128, 512], F32, name="omT_ps", tag="omT_ps")
            nc.vector.memset(omT_ps[:], 0.0)
            for ch in range(16):
                nc.tensor.transpose(omT_ps[:, ch * 32: ch * 32 + 27],
                                    om_sb[:, ch * 128:(ch + 1) * 128],
                                    id32[0:27, 0:27])
            nc.vector.tensor_copy(omT[:],
                                  omT_ps[:].rearrange("p (a b) -> p a b", a=16))

        # ------------- sample math -------------
        ppx = ph1.tile([128, 16, 32], F32)
        nc.vector.tensor_tensor(out=ppx[:], in0=omT[:], in1=base_t[:], op=ALU.add)
        ii = ph1.tile([128, 16, 18], I32)
        nc.vector.tensor_copy(ii[:], ppx[:, :, 0:18])
        ff = ph1.tile([128, 16, 18], F32)
        nc.vector.tensor_copy(ff[:], ii[:])
        gtt = ph1.tile([128, 16, 18], F32)
        nc.vector.tensor_tensor(out=gtt[:], in0=ff[:], in1=ppx[:, :, 0:18], op=ALU.is_gt)
        flo = ph1.tile([128, 16, 18], F32)
        nc.vector.tensor_tensor(out=flo[:], in0=ff[:], in1=gtt[:], op=ALU.subtract)
        lf = ph1.tile([128, 16, 18], F32)
        nc.vector.tensor_tensor(out=lf[:], in0=ppx[:, :, 0:18], in1=flo[:], op=ALU.subtract)
        floc = ph1.tile([128, 16, 18], F32)
        nc.vector.tensor_scalar(floc[:], flo[:], 0.0, float(W2 - 2), ALU.max, ALU.min)
        msk = ph1.tile([128, 16, 9], F32)
        nc.scalar.activation(msk[:], omT[:, :, 18:27], AF.Sigmoid)
        ol = ph1.tile([128, 16, 18], F32)
        nc.vector.tensor_scalar(ol[:], lf[:], -1.0, 1.0, ALU.mult, ALU.add)
        # corner weights (with mask folded), fp16: [128, 16, 9] each,
        # order (y,x) = (0,0),(0,1),(1,0),(1,1)
        w4 = []
        for r, (ya, xa) in enumerate([(0, 0), (0, 1), (1, 0), (1, 1)]):
            yw = ol if ya == 0 else lf     # (1-ly) or ly
            xw = ol if xa == 0 else lf
            wtile = ph1.tile([128, 16, 9], F32, name=f"wr4_{r}", tag=f"wr4_{r}")
            nc.vector.tensor_tensor(out=wtile[:], in0=yw[:, :, 0:9],
                                    in1=xw[:, :, 9:18], op=ALU.mult)
            nc.vector.tensor_tensor(out=wtile[:], in0=wtile[:], in1=msk[:], op=ALU.mult)
            w16 = pers.tile([128, 16, 9], F16, name=f"w4_{r}", tag=f"w4_{r}")
            nc.vector.tensor_copy(w16[:], wtile[:])
            w4.append(w16)
        # quad-gather index q = y0c*W2 + x0c + QSH (alloc rows shifted +QSH)
        qf = ph1.tile([128, 16, 9], F32)
        nc.vector.tensor_scalar(qf[:], floc[:, :, 0:9], float(W2), None, ALU.mult)
        nc.vector.scalar_tensor_tensor(out=qf[:], in0=floc[:, :, 9:18],
                                       scalar=float(QSH), in1=qf[:],
                                       op0=ALU.add, op1=ALU.add)

        # per tap: transpose [128 p, 16 ch] -> [16, 128] -> permute+cast
        idxT16 = ph1.tile([16, K * 128], I16)
        with tc.tile_pool(name="idxps", bufs=3, space="PSUM") as idxps:
            for k in range(K):
                tps = idxps.tile([16, 128], F32, name=f"tps{k}", tag="tps")
                nc.tensor.transpose(tps[:], qf[:, :, k], id32[:])
                # permuting cast copy: dst[., q*8+jj] = src[., jj*16+q]
                src = tps[:].rearrange("p (jj q) -> p q jj", jj=8)
                dst = idxT16[:, k * 128:(k + 1) * 128].rearrange(
                    "p (q jj) -> p q jj", q=16)
                nc.vector.tensor_copy(dst, src)

        # hop1: SBUF -> DRAM wrapped layout; hop2: DRAM -> SBUF + replicate
        for k in range(K):
            for sblk in range(NSB):
                dstd = idxs_t[sblk, k].rearrange(
                    "q (j jj) -> q j jj", j=8).transpose([1, 0, 2])
                src = idxT16[sblk * 8: sblk * 8 + 8,
                             k * 128:(k + 1) * 128].rearrange(
                    "j (q jj) -> j q jj", q=16)
                nc.sync.dma_start(out=dstd, in_=src)
        wrapped = pers.tile([128, NG * 64], I16, name="wrapped", tag="wrapped")
        for g in range(8):
            dst = wrapped[g * 16:(g + 1) * 16, :].rearrange(
                "q (s k f) -> q s k f", s=NSB, k=K)
            nc.sync.dma_start(out=dst, in_=idxs_t[:].transpose([2, 0, 1, 3]))

        # ------------- xt1q build (quad rows) -------------
        # quad alloc row a = true rows [a-75, a-74, a-1, a] in col blocks
        # 0..3; i.e. true row q lands at alloc rows q+75, q+74, q+1, q.
        with tc.tile_pool(name="xtp", bufs=2) as xtp, \
             tc.tile_pool(name="xtps", bufs=2, space="PSUM") as xtps:
            for grp in range(11):          # 4 q-chunks per group, 43 chunks
                qcs = range(grp * 4, min(grp * 4 + 4, 43))
                pt = xtps.tile([128, 1024], F16, name=f"xt_ps{grp}", tag="xt_ps")
                for i, qc in enumerate(qcs):
                    for cc in range(CC):
                        nc.tensor.transpose(
                            pt[:, i * 256 + cc * 128: i * 256 + (cc + 1) * 128],
                            xpad[cc][:, qc * 128:(qc + 1) * 128], id16[:])
                st = xtp.tile([128, 1024], F16, name=f"xt_sb{grp}", tag="xt_sb")
                nqc = len(list(qcs))
                nc.scalar.activation(st[:, 0:nqc * 256], pt[:, 0:nqc * 256], AF.Copy)
                src = st[:, 0:nqc * 256].rearrange("p (qc c) -> p qc c", c=256)
                r0 = grp * 512
                for rr, off in enumerate([QSH, QSH - 1, 1, 0]):
                    dst = xt1q_t[r0 + off: r0 + off + nqc * 128,
                                 rr * 256:(rr + 1) * 256].rearrange(
                        "(qc p) c -> p qc c", p=128)
                    nc.scalar.dma_start(out=dst, in_=src)

        # ------------- main loop -------------
        ph1_cm.__exit__(None, None, None)
        out_osb = [pers.tile([128, S], F32, name=f"out_osb{occ}", tag=f"oo{occ}")
                   for occ in range(OCC)]
        with tc.tile_pool(name="mg", bufs=3) as mg, \
             tc.tile_pool(name="mv", bufs=3) as mv, \
             tc.tile_pool(name="mvs", bufs=1) as mvs, \
             tc.tile_pool(name="mps", bufs=2, space="PSUM") as mps, \
             tc.tile_pool(name="accp", bufs=2, space="PSUM") as accp, \
             tc.tile_pool(name="outp", bufs=2) as outp, \
             tc.tile_pool(name="outps", bufs=2, space="PSUM") as outps:
            for sblk in range(NSB):
                ch0 = sblk * CHB
                vsb = [[None] * CC for _ in range(K)]
                for k in range(K):
                    cT = sblk * K + k
                    gt = mg.tile([128, CHB, 1024], F16, name=f"g{sblk}_{k}", tag="gt")
                    nc.gpsimd.dma_gather(gt[:], xt1q_t[:, :],
                                         wrapped[:, cT * 64:(cT + 1) * 64],
                                         NI, NI, 1024, single_packet=True,
                                         queue_num=cT % NQUEUES)
                    # wide 4-corner combine, broadcast weights over channels
                    vt = mv.tile([128, CHB, 256], F16, name=f"v{sblk}_{k}", tag="vt")
                    tmp = mv.tile([128, CHB, 256], F16, name=f"t{sblk}_{k}", tag="tmp")

                    def wb(r):
                        return w4[r][:, ch0:ch0 + CHB, k:k + 1].to_broadcast(
                            (128, CHB, 256))
                    # quad col blocks: 0 -> (y0,x0), 1 -> (y0,x1), 2 -> (y1,x0),
                    # 3 -> (y1,x1)  [true rows a-75, a-74, a-1, a]
                    nc.vector.tensor_tensor(out=vt[:], in0=gt[:, :, 0:256],
                                            in1=wb(0), op=ALU.mult)
                    nc.vector.tensor_tensor(out=tmp[:], in0=gt[:, :, 256:512],
                                            in1=wb(1), op=ALU.mult)
                    nc.vector.tensor_tensor(out=vt[:], in0=vt[:], in1=tmp[:],
                                            op=ALU.add)
                    nc.vector.tensor_tensor(out=tmp[:], in0=gt[:, :, 512:768],
                                            in1=wb(2), op=ALU.mult)
                    nc.vector.tensor_tensor(out=vt[:], in0=vt[:], in1=tmp[:],
                                            op=ALU.add)
                    nc.vector.tensor_tensor(out=tmp[:], in0=gt[:, :, 768:1024],
                                            in1=wb(3), op=ALU.mult)
                    nc.vector.tensor_tensor(out=vt[:], in0=vt[:], in1=tmp[:],
                                            op=ALU.add)
                    # transpose V^T -> V [c, s]
                    vps = [mps.tile([128, 1024], F16, name=f"vps{sblk}_{k}_{cc}",
                                    tag=f"vps{cc}") for cc in range(CC)]
                    for chp in range(CHB):
                        for cc in range(CC):
                            nc.tensor.transpose(
                                vps[cc][:, chp * 128:(chp + 1) * 128],
                                vt[:, chp, cc * 128:(cc + 1) * 128],
                                id16[:])
                    for cc in range(CC):
                        t = mvs.tile([128, 1024], F16, name=f"vsb{sblk}_{k}_{cc}",
                                     tag=f"vsb{k}_{cc}")
                        nc.scalar.activation(t[:], vps[cc][:], AF.Copy)
                        vsb[k][cc] = t
                # matmuls: per s-chunk, own PSUM bank, accumulate over (k, cc)
                for chp in range(CHB):
                    ch = sblk * CHB + chp
                    acc = accp.tile([128, O], F32, name=f"acc{sblk}_{chp}",
                                    tag="acc")
                    for k in range(K):
                        for cc in range(CC):
                            nc.tensor.matmul(
                                acc[:],
                                vsb[k][cc][:, chp * 128:(chp + 1) * 128],
                                wmain[cc][:, k * O:(k + 1) * O],
                                start=(k == 0 and cc == 0), stop=False)
                    nc.tensor.matmul(acc[:], one16[:], bprow16[:],
                                     start=False, stop=True)
                    relu = outp.tile([128, O], F32, name=f"relu{sblk}_{chp}",
                                     tag="relu")
                    nc.scalar.activation(relu[:], acc[:], AF.Relu)
                    ops_ = outps.tile([128, 256], F32, name=f"ops{sblk}_{chp}",
                                      tag="ops")
                    for occ in range(OCC):
                        nc.tensor.transpose(
                            ops_[:, occ * 128:(occ + 1) * 128],
                            relu[:, occ * 128:(occ + 1) * 128],
                            id32[:])
                    for occ in range(OCC):
                        nc.vector.tensor_copy(
                            out_osb[occ][:, ch * 128:(ch + 1) * 128],
                            ops_[:, occ * 128:(occ + 1) * 128])
        for occ in range(OCC):
            nc.sync.dma_start(out=out_d.ap()[occ * 128:(occ + 1) * 128, :],
                              in_=out_osb[occ][:])


# ===================== host side =====================

def _host_prep(inputs):
    """Build the 8 per-core input maps (layout-only host work + constants)."""
    x = np.ascontiguousarray(inputs["x"], dtype=np.float32)
    w_off = np.asarray(inputs["w_off"], np.float32)
    b_off = np.asarray(inputs["b_off"], np.float32)
    weight = np.asarray(inputs["weight"], np.float32)
    bias = np.asarray(inputs["bias"], np.float32)
    gamma = np.asarray(inputs["gamma"], np.float32)
    beta = np.asarray(inputs["beta"], np.float32)
    run_mean = np.asarray(inputs["run_mean"], np.float32)
    run_var = np.asarray(inputs["run_var"], np.float32)

    # weight [O, C, 3, 3] -> [CC, 128c, K, O] -> [CC, 128, K*O]
    wt = weight.reshape(O, C, K).transpose(1, 2, 0).reshape(CC, 128, K * O)
    wt = np.ascontiguousarray(wt)
    # w_off [27, C, 3, 3] -> [CC, 128c, K, 27]
    wofft = w_off.reshape(27, C, K).transpose(1, 2, 0).reshape(CC, 128, K * 27)
    wofft = np.ascontiguousarray(wofft)
    bnv = np.concatenate([gamma, beta, run_mean, run_var, bias]).astype(np.float32).reshape(1, 5 * O)
    id32 = np.eye(128, dtype=np.float32)
    id16 = np.eye(128, dtype=np.float16)
    ones = np.ones((1, 128), np.float16)
    boff = b_off.reshape(27, 1).astype(np.float32)

    in_maps = []
    for core in range(N_CORES):
        b, half = core // 2, core % 2
        h0 = half * HH
        # halo rows [h0-1, h0+33) with zero pad at the image boundary
        halo = np.zeros((C, 34, W), np.float32)
        lo, hi = h0 - 1, h0 + 33
        slo, shi = max(lo, 0), min(hi, H)
        halo[:, slo - lo: slo - lo + (shi - slo)] = x[b, :, slo:shi]
        # baseC [128, 16, 32]: cols 0-8 pyP base, 9-17 pxP base, rest 0
        basec = np.zeros((128, 16, 32), np.float32)
        pp_ = np.arange(128)
        for ch in range(16):
            s_ = ch * 128 + pp_
            hloc = h0 + s_ // W
            wloc = s_ % W
            for k in range(K):
                basec[:, ch, k] = hloc + (k // 3) - 1 + P
                basec[:, ch, 9 + k] = wloc + (k % 3) - 1 + P
        in_maps.append({
            "x_b": np.ascontiguousarray(x[b]),
            "xhalo": halo,
            "w_t": wt,
            "woff_t": wofft,
            "b_off": boff,
            "bnvec": bnv,
            "baseC": basec.reshape(128, 16 * 32),
            "ident32": id32,
            "ident16": id16,
            "ones16": ones,
        })
    return in_maps


def _get_nc():
    if "nc" not in _NC_CACHE:
        _NC_CACHE["nc"] = build_nc()
    return _NC_CACHE["nc"]


def kernel(**inputs):
    nc = _get_nc()
    in_maps = _host_prep(inputs)
    res = bass_utils.run_bass_kernel_spmd(nc, in_maps, core_ids=list(range(N_CORES)))
    out = np.zeros((B, O, H, W), np.float32)
    for core in range(N_CORES):
        b, half = core // 2, core % 2
        out[b, :, half * HH:(half + 1) * HH, :] = (
            res.results[core]["out_c"].reshape(O, HH, W))
    return out
